# revision 12
# baseline (speedup 1.0000x reference)
"""Trainium2 Bass kernel VB: collective-free restructure.

out = softmax((x M x^T) * tril) @ V with V = x Wv^T is computed as
  U^T[d, q] = sum_{k<128*wsc(g)} x[k, d] * P^T[k, q]  +  xsg_g[d]
  out[q, e] = (U @ Wv^T)[q, e] / den[q]
i.e. P is contracted against the raw input x (P@x), then ONE projection
through Wv^T produces the output. The V tensor never materializes, so
the V AllGather (CC stream: ~20us bootstrap barrier + 6-24us per op +
run-to-run variance) disappears entirely. Costs ~7us more tensor time
than the V-exchange scheme (PXT runs at N=256 vs PV's N=512), buys a
collective-free, variance-free schedule.

The in-window entries of P beyond a slot's own causal window are exactly
exp(0)=1 (multiplicative mask), so the host suffix xsg_g = sum_{k >=
128*wsc(g)} x[k,:] is shared by both slots of a score group.
"""

import os
import sys

sys.path.insert(0, "/opt/trn_rl_repo")

import numpy as np
import ml_dtypes

import concourse.bass as bass
import concourse.tile as tile
from concourse import bacc, mybir
from concourse import bass_utils

bass_utils.upload_artifacts = lambda tmpdir: "local://" + tmpdir

B, T, D = 4, 2048, 1024
N_CORES = 8
NDT = D // 128
NKT_ALL = T // 128

SCALE = 1.0 / float(np.sqrt(np.float32(T)))

BF = mybir.dt.bfloat16
F8 = mybir.dt.float8e4
F32 = mybir.dt.float32
bf16 = ml_dtypes.bfloat16
f8e4 = ml_dtypes.float8_e4m3

YSC2 = 512.0

_cache = {}
LAST_RESULT = None


def _w(o):          # denominator window (k-tiles) for owned subtile slot o
    return 2 * o + 2


def _wsc(g):        # score window (k-tiles) for score group g
    return 4 * g + 4


def _build():
    nc = bacc.Bacc("TRN2", target_bir_lowering=False, debug=False, num_devices=N_CORES)

    xT_d = nc.dram_tensor("xT", [128, NDT, T], F8, kind="ExternalInput")
    xn_d = nc.dram_tensor("xn", [128, NKT_ALL, D], BF, kind="ExternalInput")
    xTq_d = nc.dram_tensor("xTq", [128, NDT, 1024], F8, kind="ExternalInput")
    M_d = nc.dram_tensor("M", [128, NDT, D], F8, kind="ExternalInput")
    wv_d = nc.dram_tensor("wv", [128, NDT, D], BF, kind="ExternalInput")
    qmi_d = nc.dram_tensor("qmi", [128, 4, 512], F32, kind="ExternalInput")
    xsg_d = nc.dram_tensor("xsg", [128, NDT, 2], F32, kind="ExternalInput")
    out_d = nc.dram_tensor("out", [1024, D], BF, kind="ExternalOutput")

    xT_ap = xT_d.ap()
    xn_ap = xn_d.ap()
    xTq = xTq_d.ap()
    out_ap = out_d.ap()

    Exp = mybir.ActivationFunctionType.Exp

    with tile.TileContext(nc) as tc:
        with (
            tc.tile_pool(name="actpool", bufs=1) as actpool,
            tc.tile_pool(name="cpool", bufs=1) as cpool,
            tc.tile_pool(name="ps_big", bufs=6, space="PSUM") as ps_big,
            tc.tile_pool(name="ps_small", bufs=2, space="PSUM") as ps_small,
        ):
            ones_col = cpool.tile([128, 1], BF)
            nc.vector.memset(ones_col[:], 1.0)
            one11 = cpool.tile([1, 1], F32)
            nc.vector.memset(one11[:], 1.0)
            warm = cpool.tile([128, 512], BF)
            nc.vector.memset(warm[:], 0.000488)

            qmi = cpool.tile([128, 4, 512], F32)
            xsg = cpool.tile([128, NDT, 2], F32)

            xT = actpool.tile([128, NDT, T], F8, tag="xt")
            yT = actpool.tile([128, NDT, 1024], F8, tag="yt")
            xn = actpool.tile([128, NKT_ALL, D], BF, tag="xn")
            UT = actpool.tile([128, NDT, 1024], BF, tag="ut")
            # one 16-kt-wide P^T tile for all 4 score groups; the
            # regions a pair's window covers beyond a group's own window
            # are exactly exp(0)=1 (fully-masked tiles), memset once below
            pT_all = actpool.tile([128, NKT_ALL, 1024], BF, tag="pt")
            nc.vector.memset(pT_all[:, 4:8, 0:256], 1.0)
            nc.vector.memset(pT_all[:, 12:16, 512:768], 1.0)

            with (
                tc.tile_pool(name="xpool", bufs=1) as xpool,
                tc.tile_pool(name="wpool", bufs=1) as wpool,
                tc.tile_pool(name="mpool", bufs=3) as mpool,
                tc.tile_pool(name="spool", bufs=2) as spool,
                tc.tile_pool(name="opool", bufs=3) as opool,
            ):
                wv_t = wpool.tile([128, NDT, D], BF, tag="wv")
                Mt = wpool.tile([128, NDT, D], F8, tag="m")
                xq_t = xpool.tile([128, NDT, 1024], F8, tag="xq")

                # ---- input DMAs: fine-grained, strict need-order ----
                # per-instruction DMA streams run at ~65GB/s and round-robin,
                # so issue order == need order, alternating sync/scalar (plus
                # two early pieces on gpsimd), ~0.25-0.5MB pieces.
                # wave 1: yT inputs gate the first real chain (~11us)
                nc.sync.dma_start(Mt[:, 0:4, 0:512], M_d.ap()[:, 0:4, 0:512])
                nc.scalar.dma_start(Mt[:, 4:8, 0:512], M_d.ap()[:, 4:8, 0:512])
                nc.gpsimd.dma_start(xq_t[:, 0:4, 0:512], xTq[:, 0:4, 0:512])
                nc.gpsimd.dma_start(xq_t[:, 4:8, 0:512], xTq[:, 4:8, 0:512])
                nc.sync.dma_start(Mt[:, :, 512:1024], M_d.ap()[:, :, 512:1024])
                nc.scalar.dma_start(xq_t[:, :, 512:1024], xTq[:, :, 512:1024])
                # wave 2: scores g0+g1 rhs columns (needed ~22-28us)
                nc.sync.dma_start(xT[:, :, 0:512], xT_ap[:, :, 0:512])
                nc.scalar.dma_start(xT[:, :, 512:1024], xT_ap[:, :, 512:1024])
                # wave 3: PXT pair-0 lhs (xn kt 0-7, needed ~33us)
                nc.sync.dma_start(xn[:, 0:4, :], xn_ap[:, 0:4, :])
                nc.scalar.dma_start(xn[:, 4:8, :], xn_ap[:, 4:8, :])
                # wave 4: UW weights (needed ~45us)
                for d2 in range(2):
                    sl = slice(4 * d2, 4 * (d2 + 1))
                    nc.sync.dma_start(wv_t[:, sl, 0:512], wv_d.ap()[:, sl, 0:512])
                    nc.scalar.dma_start(
                        wv_t[:, sl, 512:1024], wv_d.ap()[:, sl, 512:1024]
                    )
                # wave 5: scores g2+g3 columns, PXT pair-1 lhs
                nc.sync.dma_start(xT[:, :, 1024:1536], xT_ap[:, :, 1024:1536])
                nc.scalar.dma_start(xT[:, :, 1536:2048], xT_ap[:, :, 1536:2048])
                nc.sync.dma_start(xn[:, 8:12, :], xn_ap[:, 8:12, :])
                nc.scalar.dma_start(xn[:, 12:16, :], xn_ap[:, 12:16, :])
                # tiny late loads on gpsimd (keeps masks unblocked early)
                nc.gpsimd.dma_start(qmi[:, :, :], qmi_d.ap()[:, :, :])
                nc.gpsimd.dma_start(xsg[:, :, :], xsg_d.ap()[:, :, :])

                # PE-clock warmup
                for wi in range(6):
                    wps = ps_big.tile([128, 512], F32, tag="big", name="wps")
                    nc.tensor.matmul(
                        wps[:], warm[:, 0:128], warm[:], start=True, stop=True
                    )

                def yt_half(c):
                    for et in range(NDT):
                        ps = ps_big.tile([128, 512], F32, tag="big", name="ps")
                        for d2 in range(NDT // 2):
                            nc.tensor.matmul(
                                ps[:],
                                Mt[:, 2 * d2 : 2 * d2 + 2, 128 * et : 128 * (et + 1)],
                                xq_t[:, 2 * d2 : 2 * d2 + 2, 512 * c : 512 * (c + 1)],
                                start=(d2 == 0),
                                stop=(d2 == NDT // 2 - 1),
                                perf_mode=mybir.MatmulPerfMode.DoubleRow,
                            )
                        nc.vector.tensor_copy(yT[:, et, 512 * c : 512 * (c + 1)], ps[:])

                rcols = {}

                def scores_group(g):
                    for kt in range(_wsc(g)):
                        zpsA = ps_big.tile([128, 512], F32, tag="big", name="zps")
                        zps = zpsA[:, 0:256]
                        for d2 in range(NDT // 2):
                            nc.tensor.matmul(
                                zps,
                                xT[:, 2 * d2 : 2 * d2 + 2, 128 * kt : 128 * (kt + 1)],
                                yT[:, 2 * d2 : 2 * d2 + 2, 256 * g : 256 * (g + 1)],
                                start=(d2 == 0),
                                stop=(d2 == NDT // 2 - 1),
                                perf_mode=mybir.MatmulPerfMode.DoubleRow,
                            )
                        if kt >= 4 * g:
                            mt = mpool.tile([128, 256], F32, tag="mask", name="mt")
                            nc.vector.tensor_scalar(
                                mt[:],
                                qmi[:, g, 0:256],
                                float(128 * kt),
                                None,
                                op0=mybir.AluOpType.is_ge,
                            )
                            nc.vector.tensor_mul(zps, zps, mt[:])
                        nc.scalar.activation(
                            pT_all[:, kt, 256 * g : 256 * (g + 1)],
                            zps,
                            Exp,
                            scale=1.0 / YSC2,
                        )

                def den_rcol(o):
                    w = _w(o)
                    dps = ps_small.tile([1, 512], F32, tag="small", name="dps", bufs=1)
                    chunks = [(s, min(4, w - s)) for s in range(0, w, 4)]
                    for ci, (s, nk) in enumerate(chunks):
                        nc.tensor.matmul(
                            dps[0:1, 0 : 128 * nk],
                            ones_col[:],
                            pT_all[:, s : s + nk, 128 * o : 128 * (o + 1)],
                            start=(ci == 0),
                            stop=(ci == len(chunks) - 1),
                        )
                    nv = 128 * min(w, 4)
                    dsb = spool.tile([1, 512], F32, tag="dsb", name="dsb")
                    nc.vector.tensor_copy(dsb[0:1, 0:nv], dps[0:1, 0:nv])
                    t2 = spool.tile([1, 128], F32, tag="t2", name="t2")
                    if w == 2:
                        nc.vector.tensor_add(t2[:], dsb[0:1, 0:128], dsb[0:1, 128:256])
                    else:
                        t1 = spool.tile([1, 256], F32, tag="t1", name="t1")
                        nc.vector.tensor_add(t1[:], dsb[0:1, 0:256], dsb[0:1, 256:512])
                        nc.vector.tensor_add(t2[:], t1[0:1, 0:128], t1[0:1, 128:256])
                    drow = spool.tile([1, 128], F32, tag="drow", name="drow")
                    nc.vector.tensor_scalar_add(drow[:], t2[:], float(T - 128 * w))
                    rrow = spool.tile([1, 128], F32, tag="rrow", name="rrow")
                    nc.vector.reciprocal(rrow[:], drow[:])
                    rps = ps_small.tile([128, 1], F32, tag="rden", name="rps", bufs=1)
                    nc.tensor.matmul(rps[:], rrow[0:1, :], one11[:])
                    rcol = spool.tile([128, 1], F32, tag="rcol", name="rcol", bufs=8)
                    nc.vector.tensor_copy(rcol[:], rps[:])
                    rcols[o] = rcol

                def pxt_pair(P):
                    # N=512 pair of score groups: bf16 weight loads only
                    # hide under 512-cycle streams (N=256 ran at half rate)
                    w = 8 if P == 0 else 16
                    for dr in range(NDT):
                        ps = ps_big.tile([128, 512], F32, tag="big", name="ups")
                        for kt in range(w):
                            nc.tensor.matmul(
                                ps[:],
                                xn[:, kt, 128 * dr : 128 * (dr + 1)],
                                pT_all[:, kt, 512 * P : 512 * (P + 1)],
                                start=(kt == 0),
                                stop=(kt == w - 1),
                            )
                        # suffix colsum folded in as a per-partition bias on
                        # the PSUM->SBUF copy; runs on the scalar engine
                        # (idle during PXT) so vector stays off the critical
                        # path and the K=1 suffix matmul disappears
                        nc.scalar.activation(
                            UT[:, dr, 512 * P : 512 * (P + 1)],
                            ps[:],
                            mybir.ActivationFunctionType.Identity,
                            bias=xsg[:, dr, P : P + 1],
                        )

                def uw_o(o):
                    ot = opool.tile([128, 1024], BF, tag="out", name="ot")
                    for ec in range(2):
                        nps = ps_big.tile([128, 512], F32, tag="big", name="nps")
                        for dr in range(NDT):
                            nc.tensor.matmul(
                                nps[:],
                                UT[:, dr, 128 * o : 128 * (o + 1)],
                                wv_t[:, dr, 512 * ec : 512 * (ec + 1)],
                                start=(dr == 0),
                                stop=(dr == NDT - 1),
                            )
                        nc.vector.tensor_scalar_mul(
                            ot[:, 512 * ec : 512 * (ec + 1)], nps[:], rcols[o][:]
                        )
                    nc.scalar.dma_start(out_ap[128 * o : 128 * (o + 1), :], ot[:])

                yt_half(0)
                scores_group(0)
                den_rcol(0)
                den_rcol(1)
                scores_group(1)
                den_rcol(2)
                den_rcol(3)
                pxt_pair(0)
                uw_o(0)
                uw_o(1)
                uw_o(2)
                yt_half(1)
                uw_o(3)
                scores_group(2)
                den_rcol(4)
                den_rcol(5)
                scores_group(3)
                den_rcol(6)
                den_rcol(7)
                pxt_pair(1)
                uw_o(4)
                uw_o(5)
                uw_o(6)
                uw_o(7)

    nc.compile()
    return nc


def get_nc():
    if "nc" not in _cache:
        _cache["nc"] = _build()
    return _cache["nc"]


def _swz(a, nt, w, dty):
    return np.ascontiguousarray(a.reshape(nt, 128, w).transpose(1, 0, 2)).astype(dty)


def make_in_maps(x, Wq, Wk, Wv):
    x = np.asarray(x, np.float32)
    Wq32 = np.asarray(Wq, np.float32)
    Wk32 = np.asarray(Wk, np.float32)
    Wv32 = np.asarray(Wv, np.float32)

    msc = SCALE * YSC2
    M_sw = _swz((Wq32.T @ Wk32) * np.float32(msc), NDT, D, f8e4)
    wv_sw = _swz(np.ascontiguousarray(Wv32.T), NDT, D, bf16)

    qmis = []
    for p in range(2):
        q = np.empty((4, 128, 512), np.float32)
        for g in range(4):
            for half in range(2):
                sub = 4 * g + 2 * half + p
                q[g, :, 128 * half : 128 * (half + 1)] = (
                    128 * sub + np.arange(128, dtype=np.float32)
                )[None, :] - np.arange(128, dtype=np.float32)[:, None]
            q[g, :, 256:512] = q[g, :, 0:256] - 128.0
        qmis.append(np.ascontiguousarray(q.transpose(1, 0, 2)))

    # xsg per batch: [128, NDT, 2] f32, col P = suffix colsum beyond the
    # PXT pair window (P=0: k >= 1024; P=1: zero)
    xsgs = []
    for b in range(B):
        rows = np.zeros((2, D), np.float32)
        rows[0] = x[b][1024:, :].sum(axis=0, dtype=np.float32)
        xsgs.append(
            np.ascontiguousarray(rows.reshape(2, NDT, 128).transpose(2, 1, 0))
        )

    in_maps = []
    for core in range(N_CORES):
        b, p = core // 2, core % 2
        xt32 = np.ascontiguousarray(x[b].T)  # [D, T] f32
        xt = _swz(xt32, NDT, T, f8e4)
        xnat = _swz(x[b], NKT_ALL, D, bf16)
        cols = [
            xt32[:, 128 * (2 * o + p) : 128 * (2 * o + p) + 128] for o in range(8)
        ]
        xq = _swz(np.ascontiguousarray(np.concatenate(cols, axis=1)), NDT, 1024, f8e4)
        in_maps.append(
            {
                "xT": xt,
                "xn": xnat,
                "xTq": xq,
                "M": M_sw,
                "wv": wv_sw,
                "qmi": qmis[p],
                "xsg": xsgs[b],
            }
        )
    return in_maps


def assemble(results):
    full = np.empty((B, T, D), np.float32)
    for core in range(N_CORES):
        b, p = core // 2, core % 2
        o_np = np.asarray(results[core]["out"], dtype=np.float32)
        for o in range(8):
            g = 2 * o + p
            full[b, 128 * g : 128 * (g + 1), :] = o_np[128 * o : 128 * (o + 1), :]
    return full


def kernel(x, Wq, Wk, Wv):
    global LAST_RESULT
    nc = get_nc()
    in_maps = make_in_maps(x, Wq, Wk, Wv)
    res = bass_utils.run_bass_kernel_spmd(nc, in_maps, core_ids=list(range(N_CORES)))
    LAST_RESULT = res
    return assemble(res.results)


# revision 13
# speedup vs baseline: 1.0094x; 1.0094x over previous
"""Trainium2 Bass kernel VB: collective-free restructure.

out = softmax((x M x^T) * tril) @ V with V = x Wv^T is computed as
  U^T[d, q] = sum_{k<128*wsc(g)} x[k, d] * P^T[k, q]  +  xsg_g[d]
  out[q, e] = (U @ Wv^T)[q, e] / den[q]
i.e. P is contracted against the raw input x (P@x), then ONE projection
through Wv^T produces the output. The V tensor never materializes, so
the V AllGather (CC stream: ~20us bootstrap barrier + 6-24us per op +
run-to-run variance) disappears entirely. Costs ~7us more tensor time
than the V-exchange scheme (PXT runs at N=256 vs PV's N=512), buys a
collective-free, variance-free schedule.

The in-window entries of P beyond a slot's own causal window are exactly
exp(0)=1 (multiplicative mask), so the host suffix xsg_g = sum_{k >=
128*wsc(g)} x[k,:] is shared by both slots of a score group.
"""

import os
import sys

sys.path.insert(0, "/opt/trn_rl_repo")

import numpy as np
import ml_dtypes

import concourse.bass as bass
import concourse.tile as tile
from concourse import bacc, mybir
from concourse import bass_utils

bass_utils.upload_artifacts = lambda tmpdir: "local://" + tmpdir

B, T, D = 4, 2048, 1024
N_CORES = 8
NDT = D // 128
NKT_ALL = T // 128

SCALE = 1.0 / float(np.sqrt(np.float32(T)))

BF = mybir.dt.bfloat16
F8 = mybir.dt.float8e4
F32 = mybir.dt.float32
bf16 = ml_dtypes.bfloat16
f8e4 = ml_dtypes.float8_e4m3

YSC2 = 512.0

_cache = {}
LAST_RESULT = None


def _w(o):          # denominator window (k-tiles) for owned subtile slot o
    return 2 * o + 2


def _wsc(g):        # score window (k-tiles) for score group g
    return 4 * g + 4


def _build():
    nc = bacc.Bacc("TRN2", target_bir_lowering=False, debug=False, num_devices=N_CORES)

    xT_d = nc.dram_tensor("xT", [128, NDT, T], F8, kind="ExternalInput")
    xn_d = nc.dram_tensor("xn", [128, NKT_ALL, D], BF, kind="ExternalInput")
    xTq_d = nc.dram_tensor("xTq", [128, NDT, 1024], F8, kind="ExternalInput")
    M_d = nc.dram_tensor("M", [128, NDT, D], F8, kind="ExternalInput")
    wv_d = nc.dram_tensor("wv", [128, NDT, D], BF, kind="ExternalInput")
    qmi_d = nc.dram_tensor("qmi", [128, 4, 512], F32, kind="ExternalInput")
    xsg_d = nc.dram_tensor("xsg", [128, NDT, 2], F32, kind="ExternalInput")
    out_d = nc.dram_tensor("out", [1024, D], BF, kind="ExternalOutput")

    xT_ap = xT_d.ap()
    xn_ap = xn_d.ap()
    xTq = xTq_d.ap()
    out_ap = out_d.ap()

    Exp = mybir.ActivationFunctionType.Exp

    with tile.TileContext(nc) as tc:
        with (
            tc.tile_pool(name="actpool", bufs=1) as actpool,
            tc.tile_pool(name="cpool", bufs=1) as cpool,
            tc.tile_pool(name="ps_big", bufs=6, space="PSUM") as ps_big,
            tc.tile_pool(name="ps_small", bufs=2, space="PSUM") as ps_small,
        ):
            ones_col = cpool.tile([128, 1], BF)
            nc.vector.memset(ones_col[:], 1.0)
            one11 = cpool.tile([1, 1], F32)
            nc.vector.memset(one11[:], 1.0)
            warm = cpool.tile([128, 512], BF)
            nc.vector.memset(warm[:], 0.000488)

            qmi = cpool.tile([128, 4, 512], F32)
            xsg = cpool.tile([128, NDT, 2], F32)

            xT = actpool.tile([128, NDT, T], F8, tag="xt")
            yT = actpool.tile([128, NDT, 1024], F8, tag="yt")
            xn = actpool.tile([128, NKT_ALL, D], BF, tag="xn")
            UT = actpool.tile([128, NDT, 1024], BF, tag="ut")
            # one 16-kt-wide P^T tile for all 4 score groups; the
            # regions a pair's window covers beyond a group's own window
            # are exactly exp(0)=1 (fully-masked tiles), memset once below
            pT_all = actpool.tile([128, NKT_ALL, 1024], BF, tag="pt")
            nc.vector.memset(pT_all[:, 4:8, 0:256], 1.0)
            nc.vector.memset(pT_all[:, 12:16, 512:768], 1.0)

            with (
                tc.tile_pool(name="xpool", bufs=1) as xpool,
                tc.tile_pool(name="wpool", bufs=1) as wpool,
                tc.tile_pool(name="mpool", bufs=3) as mpool,
                tc.tile_pool(name="spool", bufs=2) as spool,
                tc.tile_pool(name="opool", bufs=3) as opool,
            ):
                wv_t = wpool.tile([128, NDT, D], BF, tag="wv")
                Mt = wpool.tile([128, NDT, D], F8, tag="m")
                xq_t = xpool.tile([128, NDT, 1024], F8, tag="xq")

                # ---- input DMAs: fine-grained, strict need-order ----
                # per-instruction DMA streams run at ~65GB/s and round-robin,
                # so issue order == need order, alternating sync/scalar (plus
                # two early pieces on gpsimd), ~0.25-0.5MB pieces.
                # wave 1: yT inputs gate the first real chain (~11us);
                # xq col-half 1 is only needed by yt1 (~60us) so it waits
                nc.sync.dma_start(Mt[:, 0:4, 0:512], M_d.ap()[:, 0:4, 0:512])
                nc.scalar.dma_start(Mt[:, 4:8, 0:512], M_d.ap()[:, 4:8, 0:512])
                nc.gpsimd.dma_start(xq_t[:, 0:4, 0:512], xTq[:, 0:4, 0:512])
                nc.gpsimd.dma_start(xq_t[:, 4:8, 0:512], xTq[:, 4:8, 0:512])
                nc.sync.dma_start(Mt[:, :, 512:1024], M_d.ap()[:, :, 512:1024])
                # wave 2: scores g0+g1 rhs columns (needed ~22-28us)
                nc.scalar.dma_start(xT[:, :, 0:512], xT_ap[:, :, 0:512])
                nc.sync.dma_start(xT[:, :, 512:1024], xT_ap[:, :, 512:1024])
                # wave 3: PXT pair-0 lhs (xn kt 0-7, needed ~33us)
                nc.sync.dma_start(xn[:, 0:4, :], xn_ap[:, 0:4, :])
                nc.scalar.dma_start(xn[:, 4:8, :], xn_ap[:, 4:8, :])
                # wave 4: UW weights (needed ~45us)
                for d2 in range(2):
                    sl = slice(4 * d2, 4 * (d2 + 1))
                    nc.sync.dma_start(wv_t[:, sl, 0:512], wv_d.ap()[:, sl, 0:512])
                    nc.scalar.dma_start(
                        wv_t[:, sl, 512:1024], wv_d.ap()[:, sl, 512:1024]
                    )
                # wave 5: yt1 inputs, scores g2+g3 columns, PXT pair-1 lhs
                nc.scalar.dma_start(xq_t[:, :, 512:1024], xTq[:, :, 512:1024])
                nc.sync.dma_start(xT[:, :, 1024:1536], xT_ap[:, :, 1024:1536])
                nc.scalar.dma_start(xT[:, :, 1536:2048], xT_ap[:, :, 1536:2048])
                nc.sync.dma_start(xn[:, 8:12, :], xn_ap[:, 8:12, :])
                nc.scalar.dma_start(xn[:, 12:16, :], xn_ap[:, 12:16, :])
                # tiny late loads on gpsimd (keeps masks unblocked early)
                nc.gpsimd.dma_start(qmi[:, :, :], qmi_d.ap()[:, :, :])
                nc.gpsimd.dma_start(xsg[:, :, :], xsg_d.ap()[:, :, :])

                # PE-clock warmup: bridge the pre-data window with throwaway
                # matmuls so the pstate ramp is done when the first real
                # chain's inputs land (an idle gap lets the clock drop back)
                for wi in range(14):
                    wps = ps_big.tile([128, 512], F32, tag="big", name="wps")
                    nc.tensor.matmul(
                        wps[:], warm[:, 0:128], warm[:], start=True, stop=True
                    )

                def yt_half(c):
                    for et in range(NDT):
                        ps = ps_big.tile([128, 512], F32, tag="big", name="ps")
                        for d2 in range(NDT // 2):
                            nc.tensor.matmul(
                                ps[:],
                                Mt[:, 2 * d2 : 2 * d2 + 2, 128 * et : 128 * (et + 1)],
                                xq_t[:, 2 * d2 : 2 * d2 + 2, 512 * c : 512 * (c + 1)],
                                start=(d2 == 0),
                                stop=(d2 == NDT // 2 - 1),
                                perf_mode=mybir.MatmulPerfMode.DoubleRow,
                            )
                        nc.vector.tensor_copy(yT[:, et, 512 * c : 512 * (c + 1)], ps[:])

                rcols = {}

                def scores_group(g):
                    for kt in range(_wsc(g)):
                        zpsA = ps_big.tile([128, 512], F32, tag="big", name="zps")
                        zps = zpsA[:, 0:256]
                        for d2 in range(NDT // 2):
                            nc.tensor.matmul(
                                zps,
                                xT[:, 2 * d2 : 2 * d2 + 2, 128 * kt : 128 * (kt + 1)],
                                yT[:, 2 * d2 : 2 * d2 + 2, 256 * g : 256 * (g + 1)],
                                start=(d2 == 0),
                                stop=(d2 == NDT // 2 - 1),
                                perf_mode=mybir.MatmulPerfMode.DoubleRow,
                            )
                        if kt >= 4 * g:
                            mt = mpool.tile([128, 256], F32, tag="mask", name="mt")
                            nc.vector.tensor_scalar(
                                mt[:],
                                qmi[:, g, 0:256],
                                float(128 * kt),
                                None,
                                op0=mybir.AluOpType.is_ge,
                            )
                            nc.vector.tensor_mul(zps, zps, mt[:])
                        nc.scalar.activation(
                            pT_all[:, kt, 256 * g : 256 * (g + 1)],
                            zps,
                            Exp,
                            scale=1.0 / YSC2,
                        )

                def den_rcol(o):
                    w = _w(o)
                    dps = ps_small.tile([1, 512], F32, tag="small", name="dps", bufs=1)
                    chunks = [(s, min(4, w - s)) for s in range(0, w, 4)]
                    for ci, (s, nk) in enumerate(chunks):
                        nc.tensor.matmul(
                            dps[0:1, 0 : 128 * nk],
                            ones_col[:],
                            pT_all[:, s : s + nk, 128 * o : 128 * (o + 1)],
                            start=(ci == 0),
                            stop=(ci == len(chunks) - 1),
                        )
                    nv = 128 * min(w, 4)
                    dsb = spool.tile([1, 512], F32, tag="dsb", name="dsb")
                    nc.vector.tensor_copy(dsb[0:1, 0:nv], dps[0:1, 0:nv])
                    t2 = spool.tile([1, 128], F32, tag="t2", name="t2")
                    if w == 2:
                        nc.vector.tensor_add(t2[:], dsb[0:1, 0:128], dsb[0:1, 128:256])
                    else:
                        t1 = spool.tile([1, 256], F32, tag="t1", name="t1")
                        nc.vector.tensor_add(t1[:], dsb[0:1, 0:256], dsb[0:1, 256:512])
                        nc.vector.tensor_add(t2[:], t1[0:1, 0:128], t1[0:1, 128:256])
                    drow = spool.tile([1, 128], F32, tag="drow", name="drow")
                    nc.vector.tensor_scalar_add(drow[:], t2[:], float(T - 128 * w))
                    rrow = spool.tile([1, 128], F32, tag="rrow", name="rrow")
                    nc.vector.reciprocal(rrow[:], drow[:])
                    rps = ps_small.tile([128, 1], F32, tag="rden", name="rps", bufs=1)
                    nc.tensor.matmul(rps[:], rrow[0:1, :], one11[:])
                    rcol = spool.tile([128, 1], F32, tag="rcol", name="rcol", bufs=8)
                    nc.vector.tensor_copy(rcol[:], rps[:])
                    rcols[o] = rcol

                def pxt_pair(P):
                    # N=512 pair of score groups: bf16 weight loads only
                    # hide under 512-cycle streams (N=256 ran at half rate)
                    w = 8 if P == 0 else 16
                    for dr in range(NDT):
                        ps = ps_big.tile([128, 512], F32, tag="big", name="ups")
                        for kt in range(w):
                            nc.tensor.matmul(
                                ps[:],
                                xn[:, kt, 128 * dr : 128 * (dr + 1)],
                                pT_all[:, kt, 512 * P : 512 * (P + 1)],
                                start=(kt == 0),
                                stop=(kt == w - 1),
                            )
                        # suffix colsum folded in as a per-partition bias on
                        # the PSUM->SBUF copy; runs on the scalar engine
                        # (idle during PXT) so vector stays off the critical
                        # path and the K=1 suffix matmul disappears
                        nc.scalar.activation(
                            UT[:, dr, 512 * P : 512 * (P + 1)],
                            ps[:],
                            mybir.ActivationFunctionType.Identity,
                            bias=xsg[:, dr, P : P + 1],
                        )

                def uw_o(o):
                    ot = opool.tile([128, 1024], BF, tag="out", name="ot")
                    for ec in range(2):
                        nps = ps_big.tile([128, 512], F32, tag="big", name="nps")
                        for dr in range(NDT):
                            nc.tensor.matmul(
                                nps[:],
                                UT[:, dr, 128 * o : 128 * (o + 1)],
                                wv_t[:, dr, 512 * ec : 512 * (ec + 1)],
                                start=(dr == 0),
                                stop=(dr == NDT - 1),
                            )
                        nc.vector.tensor_scalar_mul(
                            ot[:, 512 * ec : 512 * (ec + 1)], nps[:], rcols[o][:]
                        )
                    nc.scalar.dma_start(out_ap[128 * o : 128 * (o + 1), :], ot[:])

                yt_half(0)
                scores_group(0)
                den_rcol(0)
                den_rcol(1)
                scores_group(1)
                den_rcol(2)
                den_rcol(3)
                pxt_pair(0)
                uw_o(0)
                uw_o(1)
                uw_o(2)
                yt_half(1)
                uw_o(3)
                scores_group(2)
                den_rcol(4)
                den_rcol(5)
                scores_group(3)
                den_rcol(6)
                den_rcol(7)
                pxt_pair(1)
                uw_o(4)
                uw_o(5)
                uw_o(6)
                uw_o(7)

    nc.compile()
    return nc


def get_nc():
    if "nc" not in _cache:
        _cache["nc"] = _build()
    return _cache["nc"]


def _swz(a, nt, w, dty):
    return np.ascontiguousarray(a.reshape(nt, 128, w).transpose(1, 0, 2)).astype(dty)


def make_in_maps(x, Wq, Wk, Wv):
    x = np.asarray(x, np.float32)
    Wq32 = np.asarray(Wq, np.float32)
    Wk32 = np.asarray(Wk, np.float32)
    Wv32 = np.asarray(Wv, np.float32)

    msc = SCALE * YSC2
    M_sw = _swz((Wq32.T @ Wk32) * np.float32(msc), NDT, D, f8e4)
    wv_sw = _swz(np.ascontiguousarray(Wv32.T), NDT, D, bf16)

    qmis = []
    for p in range(2):
        q = np.empty((4, 128, 512), np.float32)
        for g in range(4):
            for half in range(2):
                sub = 4 * g + 2 * half + p
                q[g, :, 128 * half : 128 * (half + 1)] = (
                    128 * sub + np.arange(128, dtype=np.float32)
                )[None, :] - np.arange(128, dtype=np.float32)[:, None]
            q[g, :, 256:512] = q[g, :, 0:256] - 128.0
        qmis.append(np.ascontiguousarray(q.transpose(1, 0, 2)))

    # xsg per batch: [128, NDT, 2] f32, col P = suffix colsum beyond the
    # PXT pair window (P=0: k >= 1024; P=1: zero)
    xsgs = []
    for b in range(B):
        rows = np.zeros((2, D), np.float32)
        rows[0] = x[b][1024:, :].sum(axis=0, dtype=np.float32)
        xsgs.append(
            np.ascontiguousarray(rows.reshape(2, NDT, 128).transpose(2, 1, 0))
        )

    in_maps = []
    for core in range(N_CORES):
        b, p = core // 2, core % 2
        xt32 = np.ascontiguousarray(x[b].T)  # [D, T] f32
        xt = _swz(xt32, NDT, T, f8e4)
        xnat = _swz(x[b], NKT_ALL, D, bf16)
        cols = [
            xt32[:, 128 * (2 * o + p) : 128 * (2 * o + p) + 128] for o in range(8)
        ]
        xq = _swz(np.ascontiguousarray(np.concatenate(cols, axis=1)), NDT, 1024, f8e4)
        in_maps.append(
            {
                "xT": xt,
                "xn": xnat,
                "xTq": xq,
                "M": M_sw,
                "wv": wv_sw,
                "qmi": qmis[p],
                "xsg": xsgs[b],
            }
        )
    return in_maps


def assemble(results):
    full = np.empty((B, T, D), np.float32)
    for core in range(N_CORES):
        b, p = core // 2, core % 2
        o_np = np.asarray(results[core]["out"], dtype=np.float32)
        for o in range(8):
            g = 2 * o + p
            full[b, 128 * g : 128 * (g + 1), :] = o_np[128 * o : 128 * (o + 1), :]
    return full


def kernel(x, Wq, Wk, Wv):
    global LAST_RESULT
    nc = get_nc()
    in_maps = make_in_maps(x, Wq, Wk, Wv)
    res = bass_utils.run_bass_kernel_spmd(nc, in_maps, core_ids=list(range(N_CORES)))
    LAST_RESULT = res
    return assemble(res.results)
